# revision 25
# baseline (speedup 1.0000x reference)
"""Trainium2 Bass kernel for nn_AGATCellWithMLP (B=8,N=2048,D=64,Q=16,H=2,K=8192).

Sharding: nodes_flat == arange(8192) selects flattened rows 0..8191 == batches
0..3 only; attention for batches 4..7 never reaches the output.  8 cores =
4 batches x 2 n-halves (1024 output rows each), zero collectives.

Per-core pipeline (all matmuls bf16, accumulate f32 in PSUM):
  scores computed transposed  sT[m,n] = (k@qT)  so the softmax numerator p_T
  is directly the lhsT of attn@v; rowsum via a ones-column folded into v.
  Elementwise work is spread across three engines: leaky-relu as
  max(x, 0.2x) on DVE (scalar_tensor_tensor) with a few tiles on ScalarE,
  exp on ScalarE, the adj mask multiply split DVE/Pool.
  Hypernetwork gates run transposed: out^T[o, n] accumulates
  W_q^T @ G_q with G[(c),q,n] = selT[c,n]*qv[q,n] as the moving operand
  (f=512), the qv@b bias and the c=128 augmented row folded in as one
  extra 32-row contraction tile; sigmoid/tanh are single [*,1024] passes
  and the final output is written transposed ([D, 1024]) in one DMA and
  transposed back on the host.
"""

import numpy as np

B, N, D, Q, H = 8, 2048, 64, 16, 2
C = 2 * D + 1            # 129
C8 = 16
K = 8192
NLOC = 1024              # output rows per core
NCORES = 8
NEG = -9.0e15

_GRAPH_CACHE = {}


# ----------------------------------------------------------------------------
# numpy fallback (correct for arbitrary nodes_flat; slow)
# ----------------------------------------------------------------------------
def _numpy_reference(x, h, query_vectors, adj, nodes_flat,
                     Wq, bq, Wk, bk, Wv, bv,
                     W_r, b_r, W_u, b_u, W_c, b_c):
    x = x.astype(np.float32); h = h.astype(np.float32)
    combined = np.concatenate([x, h], axis=-1)
    q = np.einsum('bnc,hcd->hbnd', combined, Wq) + bq[:, None, None, :]
    k = np.einsum('bnc,hcd->hbnd', combined, Wk) + bk[:, None, None, :]
    v = np.einsum('bnc,hcd->hbnd', combined, Wv) + bv[:, None, None, :]
    comb_attn = np.zeros((B, N, C), np.float32)
    for b in range(B):
        acc = np.zeros((N, C), np.float32)
        for hh in range(H):
            s = (q[hh, b] @ k[hh, b].T) / np.sqrt(np.float32(C8))
            s = np.where(s >= 0, s, 0.2 * s)
            s = np.where(adj[b] == 0, NEG, s)
            s = s - s.max(axis=-1, keepdims=True)
            e = np.exp(s)
            a = e / e.sum(axis=-1, keepdims=True)
            acc += a @ v[hh, b]
        comb_attn[b] = acc / H
    def gate(sel, qv, W, bb):
        t = np.einsum('kc,qco->kqo', sel, W)
        return np.einsum('kq,kqo->ko', qv, t) + qv @ bb
    sel = comb_attn.reshape(-1, C)[nodes_flat]
    r = 1.0 / (1.0 + np.exp(-gate(sel, query_vectors, W_r, b_r)))
    u = 1.0 / (1.0 + np.exp(-gate(sel, query_vectors, W_u, b_u)))
    h_flat = h.reshape(-1, D).copy()
    h_sel = r * h_flat[nodes_flat]
    h_flat[nodes_flat] = h_sel
    comb_new = np.concatenate([x, h_flat.reshape(h.shape)], axis=-1)
    sel2 = comb_new.reshape(-1, C)[nodes_flat]
    cand = np.tanh(gate(sel2, query_vectors, W_c, b_c))
    return ((1.0 - u) * h_sel + u * cand).astype(np.float32)


# ----------------------------------------------------------------------------
# Bass graph builder (one SPMD graph, shapes per core)
# ----------------------------------------------------------------------------
def _build_graph():
    import concourse.bass as bass
    import concourse.bacc as bacc
    import concourse.mybir as mybir
    import concourse.tile as tile
    from concourse import masks
    from contextlib import ExitStack

    bf = mybir.dt.bfloat16
    f32 = mybir.dt.float32
    AF = mybir.ActivationFunctionType
    MUL = mybir.AluOpType.mult
    ADD = mybir.AluOpType.add
    SUB = mybir.AluOpType.subtract
    MAX = mybir.AluOpType.max

    nc = bacc.Bacc(None, target_bir_lowering=False)

    # engine-assignment knobs (tuned against the trace):
    # S = ScalarE Prelu; P1 = DVE cast + Pool x0.2 + DVE max.
    # (walrus rejects 2-input Pool ops, so Pool only gets the scale.)
    PRELU_MODE = ['S', 'P1'] * 8

    # ---- DRAM parameters (exact SBUF layouts; one DMA each) -----------------
    wqkva_d  = nc.declare_dram_parameter("wqkva", [128, H, 2 * C8 + 130], bf, isOutput=False)
    wqkvb_d  = nc.declare_dram_parameter("wqkvb", [2, H, 2 * C8 + 130], bf, isOutput=False)
    combT_a  = nc.declare_dram_parameter("combT_a",  [128, N], bf, isOutput=False)
    combT_b  = nc.declare_dram_parameter("combT_b",  [2, N],   bf, isOutput=False)
    combTl_a = nc.declare_dram_parameter("combTl_a", [128, NLOC], bf, isOutput=False)
    combTl_b = nc.declare_dram_parameter("combTl_b", [2, NLOC],   bf, isOutput=False)
    qvT_d    = nc.declare_dram_parameter("qvT",      [Q, NLOC], bf, isOutput=False)
    hlocT_d  = nc.declare_dram_parameter("h_locT",   [D, NLOC], bf, isOutput=False)
    wru_d    = nc.declare_dram_parameter("wru_flat", [128, 16, 128], bf, isOutput=False)
    wc_d     = nc.declare_dram_parameter("wc_flat",  [128, 16, D], bf, isOutput=False)
    m2wru_d  = nc.declare_dram_parameter("m2w_ru",   [32, 128], bf, isOutput=False)
    m2wc_d   = nc.declare_dram_parameter("m2w_c",    [32, D], bf, isOutput=False)
    adjT_d   = nc.declare_dram_parameter("adjT",     [128, 16, NLOC], bf, isOutput=False)
    qvrep_d  = nc.declare_dram_parameter("qv_rep",   [128, Q, NLOC], bf, isOutput=False)
    out_d    = nc.declare_dram_parameter("out",      [D, NLOC], f32, isOutput=True)

    with ExitStack() as ctx:
        tc = ctx.enter_context(tile.TileContext(nc))
        const = ctx.enter_context(tc.tile_pool(name="const", bufs=1))

        # ---- persistent SBUF tensors + input DMAs (critical first) ----------
        wqkva = const.tile([128, H, 2 * C8 + 130], bf)   # [Wq|Wk|Wv] aug rows 0..127
        wqkvb = const.tile([2, H, 2 * C8 + 130], bf)     # aug rows 128..129
        nc.sync.dma_start(wqkva[:], wqkva_d[:])
        nc.sync.dma_start(wqkvb[:], wqkvb_d[:])
        combTa = const.tile([128, N], bf)
        combTb = const.tile([2, N], bf)
        combTla = const.tile([128, NLOC], bf)
        combTlb = const.tile([2, NLOC], bf)
        nc.sync.dma_start(combTa[:], combT_a[:])
        nc.sync.dma_start(combTb[:], combT_b[:])
        nc.sync.dma_start(combTla[:], combTl_a[:])
        nc.sync.dma_start(combTlb[:], combTl_b[:])

        # adjT and (later) the gate feature matrix G share two 16KB/partition
        # slots; qv_rep now has its own pair so its DMA can stream during
        # attention.  Both big DMAs are issued before the small gate-phase
        # parameters so the first mask tile isn't starved.
        adj_pool = ctx.enter_context(tc.tile_pool(name="adj", bufs=1))
        qvr_pool = ctx.enter_context(tc.tile_pool(name="qvr", bufs=1))
        adjT_h = []
        for bi in range(2):
            at = adj_pool.tile([128, 8, NLOC], bf, tag=f"adj{bi}")
            nc.sync.dma_start(at[:], adjT_d[:, bi * 8:(bi + 1) * 8, :])
            adjT_h.append(at)
        qvrep_h = []
        for bi in range(2):
            qr = qvr_pool.tile([128, 8, NLOC], bf, tag=f"qvr{bi}")
            nc.sync.dma_start(qr[:], qvrep_d[:, bi * 8:(bi + 1) * 8, :])
            qvrep_h.append(qr)

        # qv features for the bias/aug contraction tile: rows 0:16 get the
        # on-chip sel128-scaled copy, rows 16:32 the raw qv^T from DRAM.
        # qvT_sb is a base-0 copy for the partition-aligned DVE multiply.
        qvbig = const.tile([32, NLOC], bf)
        qvT_sb = const.tile([Q, NLOC], bf)
        nc.sync.dma_start(qvbig[16:32, :], qvT_d[:])
        nc.sync.dma_start(qvT_sb[:], qvT_d[:])
        hlocT = const.tile([D, NLOC], bf)
        nc.sync.dma_start(hlocT[:], hlocT_d[:])
        wru = const.tile([128, 16, 128], bf)
        nc.sync.dma_start(wru[:], wru_d[:])
        wc = const.tile([128, 16, D], bf)
        nc.sync.dma_start(wc[:], wc_d[:])
        m2wru = const.tile([32, 128], bf)
        nc.sync.dma_start(m2wru[:], m2wru_d[:])
        m2wc = const.tile([32, D], bf)
        nc.sync.dma_start(m2wc[:], m2wc_d[:])

        def adjT(mt):
            return adjT_h[mt // 8][:, mt % 8, :]

        def qvrep(qt):
            return qvrep_h[qt // 8][:, qt % 8, :]

        ident = const.tile([128, 128], f32)
        masks.make_identity(nc, ident[:])
        ones1 = const.tile([1, 16], bf)
        nc.gpsimd.memset(ones1[:], 1.0)

        def wslice(hh, lo, hi):
            return wqkva[:, hh, lo:hi], wqkvb[:, hh, lo:hi]

        # ---- projections: qT, kT per head; v for both heads paired ----------
        qTs = const.tile([C8, H, NLOC], bf)
        kTs = const.tile([C8, H, N], bf)
        vs  = const.tile([128, 16, H, 130], bf)

        with tc.tile_pool(name="proj_ps", bufs=2,
                          space=bass.MemorySpace.PSUM) as proj_ps:
            # q first, then k chunk-wise, so the first scores tile is
            # unblocked as early as possible
            for hh in range(H):
                wqa_, wqb_ = wslice(hh, 0, C8)
                qp = proj_ps.tile([C8, NLOC], f32, tag="kq")
                for chk in range(2):
                    sl = slice(chk * 512, (chk + 1) * 512)
                    nc.tensor.matmul(qp[:, sl], wqa_, combTla[:, sl],
                                     start=True, stop=False)
                    nc.tensor.matmul(qp[:, sl], wqb_, combTlb[:, sl],
                                     start=False, stop=True)
                nc.vector.tensor_copy(qTs[:, hh, :], qp[:])

                wka_, wkb_ = wslice(hh, C8, 2 * C8)
                for kchk in range(2):
                    kcs = slice(kchk * NLOC, (kchk + 1) * NLOC)
                    kp = proj_ps.tile([C8, NLOC], f32, tag="kq")
                    for chk in range(2):
                        sl = slice(kchk * NLOC + chk * 512,
                                   kchk * NLOC + (chk + 1) * 512)
                        dl = slice(chk * 512, (chk + 1) * 512)
                        nc.tensor.matmul(kp[:, dl], wka_, combTa[:, sl],
                                         start=True, stop=False)
                        nc.tensor.matmul(kp[:, dl], wkb_, combTb[:, sl],
                                         start=False, stop=True)
                    nc.vector.tensor_copy(kTs[:, hh, kcs], kp[:])

            # v projection, both heads in one f=260 pass per m-tile; doubles
            # as the PE warm-up burst while the big DMAs stream in.
            for mt in range(16):
                msl = slice(mt * 128, (mt + 1) * 128)
                vp = proj_ps.tile([128, H, 130], f32, tag="vp")
                nc.tensor.matmul(vp[:, :, :], combTa[:, msl],
                                 wqkva[:, :, 2 * C8:2 * C8 + 130],
                                 start=True, stop=False)
                nc.tensor.matmul(vp[:, :, :], combTb[:, msl],
                                 wqkvb[:, :, 2 * C8:2 * C8 + 130],
                                 start=False, stop=True)
                nc.vector.tensor_copy(vs[:, mt, :, :], vp[:])

        # ---- attention per head --------------------------------------------
        hp0  = const.tile([128, 8, 130], f32)     # head-0: 0.5 * h' / rowsum
        comb = const.tile([128, 8, C], f32)       # mean over heads

        pT_pool = ctx.enter_context(tc.tile_pool(name="pT", bufs=1))
        lr_pool = ctx.enter_context(tc.tile_pool(name="lr", bufs=3))
        ex_pool = ctx.enter_context(tc.tile_pool(name="ex", bufs=2))
        sm_pool = ctx.enter_context(tc.tile_pool(name="small", bufs=2))
        selT = const.tile([128, 8, 128], bf)

        attn_ctx = ExitStack()
        sc_ps = attn_ctx.enter_context(
            tc.tile_pool(name="sc_ps", bufs=2, space=bass.MemorySpace.PSUM))
        hp_ps = attn_ctx.enter_context(
            tc.tile_pool(name="hp_ps", bufs=4, space=bass.MemorySpace.PSUM))

        selT_flat = selT[:, :, :].rearrange("p a b -> p (a b)")

        # Per head, one software-pipelined loop per m-tile: scores (PE) ->
        # leaky-relu (DVE stt or ScalarE) -> exp (ScalarE) -> adj mask
        # (DVE or Pool); attn@v for tile mt-1 issues after scores of mt so
        # the PE never waits on the freshest pT.  j 4..7 attn@v runs as a
        # dense burst after the stream.
        for hh in range(H):
            pT = pT_pool.tile([128, 16, NLOC], bf, tag="pT")
            accums = []
            for _j in range(4):
                acc_t = hp_ps.tile([128, 130], f32, tag="hp")
                accums.append(acc_t)

            def attnv(mt, first, last):
                for j in range(4):
                    jsl = slice(j * 128, (j + 1) * 128)
                    nc.tensor.matmul(accums[j][:], pT[:, mt, jsl],
                                     vs[:, mt, hh, :],
                                     start=first, stop=last)

            for mp in range(8):
                # leaky-relu per m-tile (PSUM-sourced); exp + mask on pairs
                lr = lr_pool.tile([128, 2, NLOC], bf, tag="lr")
                for sub in range(2):
                    mt = 2 * mp + sub
                    msl = slice(mt * 128, (mt + 1) * 128)
                    sp = sc_ps.tile([128, NLOC], f32, tag="s")
                    for chk in range(2):
                        sl = slice(chk * 512, (chk + 1) * 512)
                        nc.tensor.matmul(sp[:, sl], kTs[:, hh, msl],
                                         qTs[:, hh, sl], start=True, stop=True)
                    mode = PRELU_MODE[mt]
                    if mode == 'S':
                        nc.scalar.activation(lr[:, sub, :], sp[:],
                                             AF.Prelu, alpha=0.2)
                    else:
                        # DVE casts PSUM->SBUF, Pool scales by 0.2 (1-input,
                        # line rate), max on DVE or Pool per the mode table
                        sb = lr_pool.tile([128, NLOC], bf, tag="sb")
                        nc.vector.tensor_copy(sb[:], sp[:])
                        t02 = lr_pool.tile([128, NLOC], bf, tag="t02")
                        nc.gpsimd.tensor_scalar(t02[:], sb[:], 0.2, None,
                                                op0=MUL)
                        nc.vector.tensor_tensor(lr[:, sub, :], t02[:], sb[:],
                                                op=MAX)
                    if mt > 0:
                        attnv(mt - 1, mt == 1, False)
                ex = ex_pool.tile([128, 2, NLOC], bf, tag="ex")
                nc.scalar.activation(ex[:, :, :], lr[:, :, :], AF.Exp)
                nc.vector.tensor_tensor(
                    pT[:, 2 * mp:2 * mp + 2, :], ex[:, :, :],
                    adjT_h[mp // 4][:, (2 * mp) % 8:(2 * mp) % 8 + 2, :],
                    op=MUL)
            attnv(15, False, True)

            def _combine(hp_ap, j):
                rs = sm_pool.tile([128, 1], f32, tag="rs")
                nc.vector.reciprocal(rs[:], hp_ap[:, 129:130])
                if hh == 0:
                    nc.vector.tensor_scalar(hp0[:, j, 0:C], hp_ap[:, 0:C],
                                            rs[:], 0.5, op0=MUL, op1=MUL)
                else:
                    t1 = sm_pool.tile([128, C], f32, tag="t1")
                    nc.vector.tensor_scalar(t1[:], hp_ap[:, 0:C], rs[:], 0.5,
                                            op0=MUL, op1=MUL)
                    nc.vector.tensor_tensor(comb[:, j, :], hp0[:, j, 0:C],
                                            t1[:], op=ADD)
                    tpj = hp_ps.tile([128, 130], f32, tag="hp")
                    nc.tensor.transpose(tpj[:, 0:128], comb[:, j, 0:128],
                                        ident[:])
                    nc.vector.tensor_copy(selT[:, j, :], tpj[:, 0:128])

            for j in range(4):
                _combine(accums[j][:], j)
            for j in range(4, 8):
                jsl = slice(j * 128, (j + 1) * 128)
                hp = hp_ps.tile([128, 130], f32, tag="hp")
                for mt in range(16):
                    nc.tensor.matmul(hp[:], pT[:, mt, jsl], vs[:, mt, hh, :],
                                     start=(mt == 0), stop=(mt == 15))
                _combine(hp[:], j)

        attn_ctx.close()

        # ---- gates (transposed): out^T[o, n] = sum_q W_q^T @ G_q ------------
        g_ps = ctx.enter_context(
            tc.tile_pool(name="g_ps", bufs=1, space=bass.MemorySpace.PSUM))
        bc_ps = ctx.enter_context(
            tc.tile_pool(name="bc_ps", bufs=1, space=bass.MemorySpace.PSUM))
        tp_ps = ctx.enter_context(
            tc.tile_pool(name="tp_ps", bufs=2, space=bass.MemorySpace.PSUM))

        s128row = const.tile([1, NLOC], bf)       # sel[:,128] as a row
        s128c   = const.tile([1, NLOC], bf)       # h_sel[:,63] as a row
        bcrow = const.tile([16, NLOC], bf)        # broadcast to 16 partitions
        ruT   = const.tile([128, NLOC], bf)       # sigmoid out: rows r | u
        uT0   = const.tile([D, NLOC], bf)         # u shifted to base 0
        hselT = const.tile([D, NLOC], bf)
        sel2T = const.tile([128, NLOC], bf)
        candT = const.tile([D, NLOC], bf)
        t_a   = const.tile([D, NLOC], bf)
        outT  = const.tile([D, NLOC], f32)

        # sel col 128 -> row via 8 tiny transposes (PSUM is free now)
        for j in range(8):
            tp = tp_ps.tile([1, 128], f32, tag="tp")
            nc.tensor.transpose(tp[:], comb[:, j, 128:129], ident[:])
            nc.vector.tensor_copy(s128row[0:1, j * 128:(j + 1) * 128], tp[:])

        def bc16(src_row):
            # broadcast a [1, NLOC] row to 16 partitions via K=1 matmuls
            bp = bc_ps.tile([16, NLOC], f32, tag="bc")
            for chk in range(2):
                sl = slice(chk * 512, (chk + 1) * 512)
                nc.tensor.matmul(bp[:, sl], ones1[:], src_row[0:1, sl],
                                 start=True, stop=True)
            nc.vector.tensor_copy(bcrow[:], bp[:])

        def gate_phase(Gsel, wmat, m2w, odim, gp):
            # qv'' = qvT * bcrow into qvbig rows 0:16 (after previous users)
            nc.vector.tensor_tensor(qvbig[0:16, :], qvT_sb[:], bcrow[:],
                                    op=MUL)
            for qt in range(Q):
                Gt = adjT_h[qt // 8][:, qt % 8, :]
                nc.vector.tensor_tensor(Gt, Gsel, qvrep(qt), op=MUL)
                for chk in range(2):
                    sl = slice(chk * 512, (chk + 1) * 512)
                    nc.tensor.matmul(gp[:, sl], wmat[:, qt, :], Gt[:, sl],
                                     start=(qt == 0), stop=False)
            for chk in range(2):
                sl = slice(chk * 512, (chk + 1) * 512)
                nc.tensor.matmul(gp[:, sl], m2w[:], qvbig[:, sl],
                                 start=False, stop=True)

        # r/u gates
        bc16(s128row)
        gp = g_ps.tile([128, NLOC], f32, tag="g")
        gate_phase(selT_flat, wru, m2wru, 128, gp)
        nc.scalar.activation(ruT[:], gp[:], AF.Sigmoid)
        nc.vector.tensor_tensor(hselT[:], ruT[0:D, :], hlocT[:], op=MUL)
        # u rows live at partitions 64..127; shift to base 0 for the output math
        nc.sync.dma_start(uT0[:], ruT[D:128, :])

        # candidate gate: sel2 = [x | r*h], col 128 handled via qv'' row
        nc.vector.tensor_copy(sel2T[0:65, :], combTla[0:65, :])
        nc.sync.dma_start(sel2T[65:128, :], hselT[0:63, :])
        nc.sync.dma_start(s128c[0:1, :], hselT[63:64, :])
        bc16(s128c)
        gc = g_ps.tile([D, NLOC], f32, tag="gc")
        gate_phase(sel2T[:, :], wc, m2wc, D, gc)
        nc.scalar.activation(candT[:], gc[:], AF.Tanh)

        # out = h_sel + u * (cand - h_sel), assembled transposed
        # (candT's tile is reused for the middle product)
        nc.vector.tensor_tensor(t_a[:], candT[:], hselT[:], op=SUB)
        nc.vector.tensor_tensor(candT[:], t_a[:], uT0[:], op=MUL)
        nc.vector.tensor_tensor(outT[:], candT[:], hselT[:], op=ADD)
        nc.sync.dma_start(out_d[:], outT[:])

    if not nc.is_finalized():
        nc.finalize()
    return nc


def _get_graph():
    if "nc" not in _GRAPH_CACHE:
        _GRAPH_CACHE["nc"] = _build_graph()
    return _GRAPH_CACHE["nc"]


# ----------------------------------------------------------------------------
# host-side input prep
# ----------------------------------------------------------------------------
def _prep_in_maps(x, h, query_vectors, adj,
                  Wq, bq, Wk, bk, Wv, bv,
                  W_r, b_r, W_u, b_u, W_c, b_c):
    import ml_dtypes
    bf = ml_dtypes.bfloat16

    scale = 1.0 / np.sqrt(np.float32(C8))

    # packed per-head augmented projection weights: [130, Wq(16)|Wk(16)|Wv2(130)]
    wqkv = np.zeros((H, 130, 2 * C8 + 130), np.float32)
    for hh in range(H):
        wqkv[hh, 0:C, 0:C8] = Wq[hh] * scale
        wqkv[hh, C, 0:C8] = bq[hh] * scale
        wqkv[hh, 0:C, C8:2 * C8] = Wk[hh]
        wqkv[hh, C, C8:2 * C8] = bk[hh]
        wqkv[hh, 0:C, 2 * C8:2 * C8 + C] = Wv[hh]
        wqkv[hh, C, 2 * C8:2 * C8 + C] = bv[hh]
        wqkv[hh, C, 2 * C8 + C] = 1.0          # ones-column -> rowsum
    wqkv = np.ascontiguousarray(wqkv.transpose(1, 0, 2))      # [130, H, 162]

    # gate weights, flattened (q-major over (q, c)) for c = 0..127,
    # reshaped to the SBUF tile layout [128(c), 16(q), outdim]
    wru_flat = np.concatenate([W_r[:, 0:128, :], W_u[:, 0:128, :]], axis=2)
    wru_flat = np.ascontiguousarray(wru_flat.transpose(1, 0, 2))  # [128, 16, 128]
    wc_flat = np.ascontiguousarray(W_c[:, 0:128, :].transpose(1, 0, 2))
    # stacked aug-row weights for the extra contraction tile:
    # rows 0:16 act on qv*sel128 (W[:,128,:]), rows 16:32 on raw qv (bias)
    m2w_ru = np.concatenate(
        [np.concatenate([W_r[:, 128, :], W_u[:, 128, :]], axis=1),
         np.concatenate([b_r, b_u], axis=1)], axis=0)             # [32, 128]
    m2w_c = np.concatenate([W_c[:, 128, :], b_c], axis=0)         # [32, 64]

    shared = {
        "wqkva": wqkv[0:128].astype(bf), "wqkvb": wqkv[128:130].astype(bf),
        "wru_flat": wru_flat.astype(bf), "wc_flat": wc_flat.astype(bf),
        "m2w_ru": m2w_ru.astype(bf), "m2w_c": m2w_c.astype(bf),
    }

    in_maps = []
    for core in range(NCORES):
        b, half = core // 2, core % 2
        n0 = half * NLOC
        g0 = b * N + n0

        combined = np.concatenate(
            [x[b], h[b], np.ones((N, 1), np.float32)], axis=1)    # [N, 130]
        combT = np.ascontiguousarray(combined.T)                  # [130, N]
        qvT = np.ascontiguousarray(query_vectors[g0:g0 + NLOC].T) # [16, 1024]
        # adjT[p, mt, k] = adj[b][n0+k, mt*128+p]
        adjT = np.ascontiguousarray(
            adj[b].T[:, n0:n0 + NLOC].reshape(16, 128, NLOC)
            .transpose(1, 0, 2)).astype(np.float32)               # [128,16,1024]
        qvrep = np.ascontiguousarray(
            np.broadcast_to(qvT[None, :, :], (128, Q, NLOC)))     # [128,16,1024]

        m = {
            "combT_a": combT[0:128].astype(bf),
            "combT_b": combT[128:130].astype(bf),
            "combTl_a": np.ascontiguousarray(combT[0:128, n0:n0 + NLOC]).astype(bf),
            "combTl_b": np.ascontiguousarray(combT[128:130, n0:n0 + NLOC]).astype(bf),
            "adjT": adjT.astype(bf),
            "qv_rep": qvrep.astype(bf),
            "qvT": qvT.astype(bf),
            "h_locT": np.ascontiguousarray(h[b, n0:n0 + NLOC].T).astype(bf),
        }
        m.update(shared)
        in_maps.append(m)
    return in_maps


# ----------------------------------------------------------------------------
# entry point
# ----------------------------------------------------------------------------
def kernel(x, h, query_vectors, adj, nodes_flat,
           Wq, bq, Wk, bk, Wv, bv,
           W_r, b_r, W_u, b_u, W_c, b_c, _trace=False):
    args = dict(x=np.asarray(x, np.float32), h=np.asarray(h, np.float32),
                query_vectors=np.asarray(query_vectors, np.float32),
                adj=np.asarray(adj), nodes_flat=np.asarray(nodes_flat),
                Wq=np.asarray(Wq, np.float32), bq=np.asarray(bq, np.float32),
                Wk=np.asarray(Wk, np.float32), bk=np.asarray(bk, np.float32),
                Wv=np.asarray(Wv, np.float32), bv=np.asarray(bv, np.float32),
                W_r=np.asarray(W_r, np.float32), b_r=np.asarray(b_r, np.float32),
                W_u=np.asarray(W_u, np.float32), b_u=np.asarray(b_u, np.float32),
                W_c=np.asarray(W_c, np.float32), b_c=np.asarray(b_c, np.float32))

    if not np.array_equal(args["nodes_flat"].ravel(),
                          np.arange(K, dtype=np.int64)):
        return _numpy_reference(**args)

    from concourse.bass_utils import run_bass_kernel_spmd

    nc = _get_graph()
    in_maps = _prep_in_maps(
        args["x"], args["h"], args["query_vectors"], args["adj"],
        args["Wq"], args["bq"], args["Wk"], args["bk"], args["Wv"], args["bv"],
        args["W_r"], args["b_r"], args["W_u"], args["b_u"],
        args["W_c"], args["b_c"])

    res = run_bass_kernel_spmd(nc, in_maps, core_ids=list(range(NCORES)),
                               trace=_trace)
    out = np.concatenate(
        [np.asarray(res.results[i]["out"], np.float32).T
         for i in range(NCORES)], axis=0)
    if _trace:
        kernel.last_exec_time_ns = res.exec_time_ns
    return out
